# revision 25
# baseline (speedup 1.0000x reference)
"""Trainium2 Bass kernel for GAT-style single-query attention.

Reference computation (N=16384, D=1024, H=8):
    scores[n,h] = leaky_relu(x0 @ Wi[h] + x[n] @ Wj[h] + b[h], 0.01)
    probs       = softmax(scores, axis=n)  (per head)
    out[d]      = relu(mean_h(sum_n probs[n,h] * x[n,d]))

Strategy: shard rows (N) across 8 cores.  Each core:
  - DMAs its [2048, 1024] shard to SBUF in 8 pipeline groups (rows laid out
    so each partition reads contiguous bursts),
  - transposes X 128x128 blocks on the PE (the scores matmul contracts over
    D, so X must be presented d-on-partitions),
  - computes scores^T [8, n] on the PE (Wj^T stationary, X^T moving), with
    the per-head constant (x0 @ Wi[h] + b[h]) folded in as a K=1 matmul row,
  - u = exp(leaky(s)) computed as max(exp(s), exp(0.01 s)) (exp monotone);
    one function table on the scalar engine, softmax denominator
    accumulated on the fly (no max-subtraction needed: scores are in
    [-9, 8] for this distribution),
  - transposes u back to natural layout; unnormalized weighted sums
    u^T @ X on the PE.
Each core ships its [H, D] partial sums + [H] denominators; the host sums
the 8 partials and finishes relu(mean_h HO_h / Z_h) during the unshard
step (66KB total — an on-device AllReduce is available behind
use_collective=True but costs ~35us in this environment).

Matmuls run as float32r (single-pass fp32, ~TF32 precision, 4x faster than
two-pass fp32); measured end-to-end error vs the f32 reference is ~2e-4
relative to output scale.
"""

import sys

sys.path.insert(0, "/opt/trn_rl_repo")

import numpy as np

import concourse.bacc as bacc
import concourse.tile as tile
from concourse import mybir
from concourse.bass_utils import run_bass_kernel_spmd

N, D, H = 16384, 1024, 8
NCORES = 8
NSHARD = N // NCORES          # 2048 rows per core
KCH = NSHARD // 128           # 16 n-chunks of 128 rows
DCH = D // 128                # 8 d-chunks of 128 cols
NGROUPS = 8                   # pipeline groups
KPG = KCH // NGROUPS          # n-chunks per group
F32 = mybir.dt.float32
BF16 = mybir.dt.bfloat16
F32R = mybir.dt.float32r
AR_W = 1032                   # 1024 head-sums + 1 denom + pad to 32B rows


def _build(use_fp32r=True, use_collective=False, id_bf16=False):
    # Tiles consumed by reduced-precision matmuls must be *produced* as
    # float32r (the BIR verifier checks the producing instruction's output
    # dtype).  The DVE copies out of PSUM do the rounding; the X DMA is a
    # byte-bitcast (hardware rounds on read).
    RED = F32R if use_fp32r else F32

    nc = bacc.Bacc("TRN2", target_bir_lowering=False, debug=False,
                   num_devices=NCORES)
    x_in = nc.dram_tensor("x", [NSHARD, D], F32, kind="ExternalInput").ap()
    w_in = nc.dram_tensor("w", [H, 2 * D], F32, kind="ExternalInput").ap()
    b_in = nc.dram_tensor("b", [1, H], F32, kind="ExternalInput").ap()
    x0_in = nc.dram_tensor("x0", [DCH, 128], F32, kind="ExternalInput").ap()
    out_t = nc.dram_tensor("out", [H, AR_W], F32, kind="ExternalOutput").ap()

    eye_f32 = nc.inline_tensor(np.eye(128, dtype=np.float32), "eye_f32")
    idt = BF16 if id_bf16 else RED
    eye_idt = nc.inline_tensor(
        np.eye(128, dtype=mybir.dt.np(idt)), "eye_idt")
    ones_npy = nc.inline_tensor(
        np.ones((1, 512), dtype=np.float32), "ones_npy")

    with tile.TileContext(nc) as tc:
        with (
            tc.tile_pool(name="consts", bufs=1) as consts,
            tc.tile_pool(name="xn", bufs=1) as xn_pool,
            tc.tile_pool(name="xt", bufs=1) as xt_pool,
            tc.tile_pool(name="small", bufs=1) as small,
            tc.tile_pool(name="pt", bufs=2, space="PSUM") as pt_pool,
            tc.tile_pool(name="pu", bufs=1, space="PSUM") as pu_pool,
            tc.tile_pool(name="pscore", bufs=2, space="PSUM") as pscore_pool,
            tc.tile_pool(name="pho", bufs=1, space="PSUM") as pho_pool,
            tc.tile_pool(name="pmisc", bufs=1, space="PSUM") as pmisc_pool,
            tc.tile_pool(name="dram", bufs=1, space="DRAM") as dram,
        ):
            # ---- constants (from NEFF-embedded tensors; no gpsimd) ----
            id8 = consts.tile([H, H], F32)
            nc.sync.dma_start(out=id8[:], in_=eye_f32.ap()[0:H, 0:H])
            idX = consts.tile([128, 128], idt)
            nc.sync.dma_start(out=idX[:], in_=eye_idt.ap().bitcast(idt))
            ones_row = consts.tile([1, 512], RED)
            nc.sync.dma_start(out=ones_row[:],
                              in_=ones_npy.ap().bitcast(RED))

            # ---- X DMAs: group 0 first so the PE can start ASAP ----
            # row layout: n = p*KCH + k  ->  each partition reads contiguous
            # bursts from HBM
            x_view = x_in.rearrange("(p k) d -> p k d", k=KCH)
            xn_tiles = []
            for g in range(NGROUPS):
                xn = xn_pool.tile([128, KPG, D], RED, tag=f"xn{g}")
                xn_tiles.append(xn)
                nc.sync.dma_start(
                    out=xn[:],
                    in_=x_view[:, g * KPG:(g + 1) * KPG, :].bitcast(RED))
                if g == 0:
                    w_sb = small.tile([H, 2 * D], F32)
                    nc.sync.dma_start(out=w_sb[:], in_=w_in[:])
                    b_sb = small.tile([1, H], F32)
                    nc.sync.dma_start(out=b_sb[:], in_=b_in[:])
                    x0t = small.tile([128, DCH], F32)
                    nc.sync.dma_start(out=x0t[:],
                                      in_=x0_in.rearrange("c p -> p c"))

            # ---- W^T chunks: wt_i (fp32, for cvec), wt_j (RED, scores) ----
            wt_i = small.tile([128, DCH, H], F32)
            wt_j = small.tile([128, DCH, H], RED)
            for half, dst in ((0, wt_i), (1, wt_j)):
                pw = pmisc_pool.tile([128, DCH, H], F32, tag="pm")
                for c in range(DCH):
                    nc.tensor.transpose(
                        pw[:, c, :],
                        w_sb[:, (half * DCH + c) * 128:(half * DCH + c + 1) * 128],
                        id8[:],
                    )
                nc.vector.tensor_copy(dst[:], pw[:])

            # ---- cvec[1, h] = x0 @ Wi[h] + b[h], as a K=1 bias row ----
            pc = pmisc_pool.tile([1, H], F32, tag="pm")
            for c in range(DCH):
                nc.tensor.matmul(pc[:], x0t[:, c:c + 1], wt_i[:, c, :],
                                 start=(c == 0), stop=(c == DCH - 1))
            cvec = small.tile([1, H], RED)
            nc.vector.tensor_tensor(cvec[:], pc[:], b_sb[:],
                                    mybir.AluOpType.add)

            # ---- main pipeline over groups ----
            u_tiles = []
            s_parts = small.tile([H, NGROUPS], F32)
            NF = KPG * 128  # free size per group

            for g in range(NGROUPS):
                xn = xn_tiles[g]
                # transpose the group's [128,128] blocks: X^T chunks
                xts = []
                for c in range(DCH):
                    ptt = pt_pool.tile([128, NF], RED, tag="pt")
                    for j in range(KPG):
                        nc.tensor.transpose(
                            ptt[:, j * 128:(j + 1) * 128],
                            xn[:, j, c * 128:(c + 1) * 128],
                            idX[:],
                        )
                    xt = xt_pool.tile([128, NF], RED, tag=f"xt{c}_{g}")
                    xts.append(xt)
                    nc.vector.tensor_copy(xt[:], ptt[:])

                # scores^T tile for this group (+ bias row, K=1)
                ps = pscore_pool.tile([H, NF], F32, tag="ps")
                for c in range(DCH):
                    nc.tensor.matmul(ps[:], wt_j[:, c, :], xts[c][:],
                                     start=(c == 0), stop=False)
                nc.tensor.matmul(ps[:], cvec[:], ones_row[:, 0:NF],
                                 start=False, stop=True)

                # u = exp(leaky(s)) = max(exp(s), exp(0.01 s)) (exp monotone)
                e1 = small.tile([H, NF], F32, tag=f"e1{g}")
                nc.scalar.activation(
                    e1[:], ps[:], mybir.ActivationFunctionType.Exp)
                e2 = small.tile([H, NF], F32, tag=f"e2{g}")
                nc.scalar.activation(
                    e2[:], ps[:], mybir.ActivationFunctionType.Exp, scale=0.01)
                u_sb = small.tile([H, NF], F32, tag=f"u{g}")
                nc.vector.scalar_tensor_tensor(
                    u_sb[:], e1[:], 1.0, e2[:],
                    mybir.AluOpType.mult, mybir.AluOpType.max,
                    accum_out=s_parts[:, g:g + 1])

                # transpose u back to natural layout [128, k, 8]
                pu = pu_pool.tile([128, KPG, H], F32, tag="pu")
                for j in range(KPG):
                    nc.tensor.transpose(
                        pu[:, j, :],
                        u_sb[:, j * 128:(j + 1) * 128],
                        id8[:],
                    )
                u_nat = small.tile([128, KPG, H], RED, tag=f"un{g}")
                u_tiles.append(u_nat)
                nc.vector.tensor_copy(u_nat[:], pu[:])

            # ---- phase 2: HO[h, d] = sum_n u[n, h] x[n, d] ----
            ho0 = pho_pool.tile([H, 512], F32, tag="ho0")
            ho1 = pho_pool.tile([H, 512], F32, tag="ho1")
            for k in range(KCH):
                g, j = divmod(k, KPG)
                first, last = (k == 0), (k == KCH - 1)
                nc.tensor.matmul(ho0[:], u_tiles[g][:, j, :],
                                 xn_tiles[g][:, j, 0:512],
                                 start=first, stop=last)
                nc.tensor.matmul(ho1[:], u_tiles[g][:, j, :],
                                 xn_tiles[g][:, j, 512:1024],
                                 start=first, stop=last)

            # ---- payload: [8, 1024 HO | 1 Z | pad] ----
            ar_sb = small.tile([H, AR_W], F32)
            nc.vector.memset(ar_sb[:, 1024:], 0.0)
            nc.vector.tensor_copy(ar_sb[:, 0:512], ho0[:])
            nc.vector.tensor_copy(ar_sb[:, 512:1024], ho1[:])
            nc.vector.tensor_reduce(ar_sb[:, 1024:1025], s_parts[:],
                                    axis=mybir.AxisListType.X,
                                    op=mybir.AluOpType.add)

            if use_collective:
                cc_in = dram.tile([H, AR_W], F32)
                cc_out = dram.tile([H, AR_W], F32)
                nc.sync.dma_start(out=cc_in[:], in_=ar_sb[:])
                nc.gpsimd.collective_compute(
                    "AllReduce",
                    mybir.AluOpType.add,
                    replica_groups=[list(range(NCORES))],
                    ins=[cc_in.opt()],
                    outs=[cc_out.opt()],
                )
                nc.sync.dma_start(out=out_t[:], in_=cc_out[:])
            else:
                # each core ships its partials; host sums during unshard
                nc.sync.dma_start(out=out_t[:], in_=ar_sb[:])

    nc.compile()
    return nc


_CACHE = {}


def _get_program(use_fp32r=True, use_collective=False, id_bf16=False):
    key = (bool(use_fp32r), bool(use_collective), bool(id_bf16))
    if key not in _CACHE:
        _CACHE[key] = _build(*key)
    return _CACHE[key]


def _in_maps(final_result, W, b):
    final_result = np.ascontiguousarray(final_result, dtype=np.float32)
    W = np.ascontiguousarray(W, dtype=np.float32)
    b = np.ascontiguousarray(b, dtype=np.float32).reshape(1, H)
    x0 = np.ascontiguousarray(final_result[0]).reshape(DCH, 128)
    return [
        {
            "x": final_result[c * NSHARD:(c + 1) * NSHARD],
            "w": W,
            "b": b,
            "x0": x0,
        }
        for c in range(NCORES)
    ]


def _finalize(ar):
    ho = ar[:, 0:D]
    z = ar[:, D:D + 1]
    r = (ho / (H * z)).sum(axis=0, dtype=np.float32)
    return np.maximum(r, np.float32(0)).astype(np.float32)


def kernel(final_result, W, b):
    nc = _get_program()
    res = run_bass_kernel_spmd(nc, _in_maps(final_result, W, b),
                               list(range(NCORES)))
    parts = [np.asarray(res.results[c]["out"], dtype=np.float32)
             for c in range(NCORES)]
    return _finalize(np.sum(parts, axis=0, dtype=np.float32))


if __name__ == "__main__":
    rng = np.random.default_rng(0)
    x = rng.standard_normal((N, D), dtype=np.float32)
    W = (rng.standard_normal((H, 2 * D)) * 0.05).astype(np.float32)
    b = (rng.standard_normal(H) * 0.05).astype(np.float32)
    out = kernel(final_result=x, W=W, b=b)
    print("kernel out:", out.shape, out[:8])


# revision 26
# speedup vs baseline: 1.0054x; 1.0054x over previous
"""Trainium2 Bass kernel for GAT-style single-query attention.

Reference computation (N=16384, D=1024, H=8):
    scores[n,h] = leaky_relu(x0 @ Wi[h] + x[n] @ Wj[h] + b[h], 0.01)
    probs       = softmax(scores, axis=n)  (per head)
    out[d]      = relu(mean_h(sum_n probs[n,h] * x[n,d]))

Strategy: shard rows (N) across 8 cores.  Each core:
  - DMAs its [2048, 1024] shard to SBUF in 8 pipeline groups (rows laid out
    so each partition reads contiguous bursts),
  - transposes X 128x128 blocks on the PE (the scores matmul contracts over
    D, so X must be presented d-on-partitions),
  - computes scores^T [8, n] on the PE (Wj^T stationary, X^T moving), with
    the per-head constant (x0 @ Wi[h] + b[h]) folded in as a K=1 matmul row,
  - u = exp(leaky(s)) computed as max(exp(s), exp(0.01 s)) (exp monotone);
    one function table on the scalar engine, softmax denominator
    accumulated on the fly (no max-subtraction needed: scores are in
    [-9, 8] for this distribution),
  - transposes u back to natural layout; unnormalized weighted sums
    u^T @ X on the PE.
Each core ships its [H, D] partial sums + [H] denominators; the host sums
the 8 partials and finishes relu(mean_h HO_h / Z_h) during the unshard
step (66KB total — an on-device AllReduce is available behind
use_collective=True but costs ~35us in this environment).

Matmuls run as float32r (single-pass fp32, ~TF32 precision, 4x faster than
two-pass fp32); measured end-to-end error vs the f32 reference is ~2e-4
relative to output scale.
"""

import sys

sys.path.insert(0, "/opt/trn_rl_repo")

import numpy as np

import concourse.bacc as bacc
import concourse.tile as tile
from concourse import mybir
from concourse.bass_utils import run_bass_kernel_spmd

N, D, H = 16384, 1024, 8
NCORES = 8
NSHARD = N // NCORES          # 2048 rows per core
KCH = NSHARD // 128           # 16 n-chunks of 128 rows
DCH = D // 128                # 8 d-chunks of 128 cols
NGROUPS = 4                   # pipeline groups
KPG = KCH // NGROUPS          # n-chunks per group
F32 = mybir.dt.float32
BF16 = mybir.dt.bfloat16
F32R = mybir.dt.float32r
AR_W = 1032                   # 1024 head-sums + 1 denom + pad to 32B rows


def _build(use_fp32r=True, use_collective=False, id_bf16=False):
    # Tiles consumed by reduced-precision matmuls must be *produced* as
    # float32r (the BIR verifier checks the producing instruction's output
    # dtype).  The DVE copies out of PSUM do the rounding; the X DMA is a
    # byte-bitcast (hardware rounds on read).
    RED = F32R if use_fp32r else F32

    nc = bacc.Bacc("TRN2", target_bir_lowering=False, debug=False,
                   num_devices=NCORES)
    x_in = nc.dram_tensor("x", [NSHARD, D], F32, kind="ExternalInput").ap()
    w_in = nc.dram_tensor("w", [H, 2 * D], F32, kind="ExternalInput").ap()
    b_in = nc.dram_tensor("b", [1, H], F32, kind="ExternalInput").ap()
    x0_in = nc.dram_tensor("x0", [DCH, 128], F32, kind="ExternalInput").ap()
    out_t = nc.dram_tensor("out", [H, AR_W], F32, kind="ExternalOutput").ap()

    eye_f32 = nc.inline_tensor(np.eye(128, dtype=np.float32), "eye_f32")
    idt = BF16 if id_bf16 else RED
    eye_idt = nc.inline_tensor(
        np.eye(128, dtype=mybir.dt.np(idt)), "eye_idt")
    ones_npy = nc.inline_tensor(
        np.ones((1, 512), dtype=np.float32), "ones_npy")

    with tile.TileContext(nc) as tc:
        with (
            tc.tile_pool(name="consts", bufs=1) as consts,
            tc.tile_pool(name="xn", bufs=1) as xn_pool,
            tc.tile_pool(name="xt", bufs=1) as xt_pool,
            tc.tile_pool(name="small", bufs=1) as small,
            tc.tile_pool(name="pt", bufs=2, space="PSUM") as pt_pool,
            tc.tile_pool(name="pu", bufs=1, space="PSUM") as pu_pool,
            tc.tile_pool(name="pscore", bufs=2, space="PSUM") as pscore_pool,
            tc.tile_pool(name="pho", bufs=1, space="PSUM") as pho_pool,
            tc.tile_pool(name="pmisc", bufs=1, space="PSUM") as pmisc_pool,
            tc.tile_pool(name="dram", bufs=1, space="DRAM") as dram,
        ):
            # ---- constants (from NEFF-embedded tensors; no gpsimd) ----
            id8 = consts.tile([H, H], F32)
            nc.sync.dma_start(out=id8[:], in_=eye_f32.ap()[0:H, 0:H])
            idX = consts.tile([128, 128], idt)
            nc.sync.dma_start(out=idX[:], in_=eye_idt.ap().bitcast(idt))
            ones_row = consts.tile([1, 512], RED)
            nc.sync.dma_start(out=ones_row[:],
                              in_=ones_npy.ap().bitcast(RED))

            # ---- X DMAs: group 0 first so the PE can start ASAP ----
            # row layout: n = p*KCH + k  ->  each partition reads contiguous
            # bursts from HBM
            x_view = x_in.rearrange("(p k) d -> p k d", k=KCH)
            xn_tiles = []
            for g in range(NGROUPS):
                xn = xn_pool.tile([128, KPG, D], RED, tag=f"xn{g}")
                xn_tiles.append(xn)
                nc.sync.dma_start(
                    out=xn[:],
                    in_=x_view[:, g * KPG:(g + 1) * KPG, :].bitcast(RED))
                if g == 0:
                    w_sb = small.tile([H, 2 * D], F32)
                    nc.sync.dma_start(out=w_sb[:], in_=w_in[:])
                    b_sb = small.tile([1, H], F32)
                    nc.sync.dma_start(out=b_sb[:], in_=b_in[:])
                    x0t = small.tile([128, DCH], F32)
                    nc.sync.dma_start(out=x0t[:],
                                      in_=x0_in.rearrange("c p -> p c"))

            # ---- W^T chunks: wt_i (fp32, for cvec), wt_j (RED, scores) ----
            wt_i = small.tile([128, DCH, H], F32)
            wt_j = small.tile([128, DCH, H], RED)
            for half, dst in ((0, wt_i), (1, wt_j)):
                pw = pmisc_pool.tile([128, DCH, H], F32, tag="pm")
                for c in range(DCH):
                    nc.tensor.transpose(
                        pw[:, c, :],
                        w_sb[:, (half * DCH + c) * 128:(half * DCH + c + 1) * 128],
                        id8[:],
                    )
                nc.vector.tensor_copy(dst[:], pw[:])

            # ---- cvec[1, h] = x0 @ Wi[h] + b[h], as a K=1 bias row ----
            pc = pmisc_pool.tile([1, H], F32, tag="pm")
            for c in range(DCH):
                nc.tensor.matmul(pc[:], x0t[:, c:c + 1], wt_i[:, c, :],
                                 start=(c == 0), stop=(c == DCH - 1))
            cvec = small.tile([1, H], RED)
            nc.vector.tensor_tensor(cvec[:], pc[:], b_sb[:],
                                    mybir.AluOpType.add)

            # ---- main pipeline over groups ----
            u_tiles = []
            s_parts = small.tile([H, NGROUPS], F32)
            NF = KPG * 128  # free size per group

            for g in range(NGROUPS):
                xn = xn_tiles[g]
                # transpose the group's [128,128] blocks: X^T chunks
                xts = []
                for c in range(DCH):
                    ptt = pt_pool.tile([128, NF], RED, tag="pt")
                    for j in range(KPG):
                        nc.tensor.transpose(
                            ptt[:, j * 128:(j + 1) * 128],
                            xn[:, j, c * 128:(c + 1) * 128],
                            idX[:],
                        )
                    xt = xt_pool.tile([128, NF], RED, tag=f"xt{c}_{g}")
                    xts.append(xt)
                    nc.vector.tensor_copy(xt[:], ptt[:])

                # scores^T tile for this group (+ bias row, K=1)
                ps = pscore_pool.tile([H, NF], F32, tag="ps")
                for c in range(DCH):
                    nc.tensor.matmul(ps[:], wt_j[:, c, :], xts[c][:],
                                     start=(c == 0), stop=False)
                nc.tensor.matmul(ps[:], cvec[:], ones_row[:, 0:NF],
                                 start=False, stop=True)

                # u = exp(leaky(s)) = max(exp(s), exp(0.01 s)) (exp monotone)
                e1 = small.tile([H, NF], F32, tag=f"e1{g}")
                nc.scalar.activation(
                    e1[:], ps[:], mybir.ActivationFunctionType.Exp)
                e2 = small.tile([H, NF], F32, tag=f"e2{g}")
                nc.scalar.activation(
                    e2[:], ps[:], mybir.ActivationFunctionType.Exp, scale=0.01)
                u_sb = small.tile([H, NF], F32, tag=f"u{g}")
                nc.vector.scalar_tensor_tensor(
                    u_sb[:], e1[:], 1.0, e2[:],
                    mybir.AluOpType.mult, mybir.AluOpType.max,
                    accum_out=s_parts[:, g:g + 1])

                # transpose u back to natural layout [128, k, 8]
                pu = pu_pool.tile([128, KPG, H], F32, tag="pu")
                for j in range(KPG):
                    nc.tensor.transpose(
                        pu[:, j, :],
                        u_sb[:, j * 128:(j + 1) * 128],
                        id8[:],
                    )
                u_nat = small.tile([128, KPG, H], RED, tag=f"un{g}")
                u_tiles.append(u_nat)
                nc.vector.tensor_copy(u_nat[:], pu[:])

            # ---- phase 2: HO[h, d] = sum_n u[n, h] x[n, d] ----
            ho0 = pho_pool.tile([H, 512], F32, tag="ho0")
            ho1 = pho_pool.tile([H, 512], F32, tag="ho1")
            for k in range(KCH):
                g, j = divmod(k, KPG)
                first, last = (k == 0), (k == KCH - 1)
                nc.tensor.matmul(ho0[:], u_tiles[g][:, j, :],
                                 xn_tiles[g][:, j, 0:512],
                                 start=first, stop=last)
                nc.tensor.matmul(ho1[:], u_tiles[g][:, j, :],
                                 xn_tiles[g][:, j, 512:1024],
                                 start=first, stop=last)

            # ---- payload: [8, 1024 HO | 1 Z | pad] ----
            ar_sb = small.tile([H, AR_W], F32)
            nc.vector.memset(ar_sb[:, 1024:], 0.0)
            nc.vector.tensor_copy(ar_sb[:, 0:512], ho0[:])
            nc.vector.tensor_copy(ar_sb[:, 512:1024], ho1[:])
            nc.vector.tensor_reduce(ar_sb[:, 1024:1025], s_parts[:],
                                    axis=mybir.AxisListType.X,
                                    op=mybir.AluOpType.add)

            if use_collective:
                cc_in = dram.tile([H, AR_W], F32)
                cc_out = dram.tile([H, AR_W], F32)
                nc.sync.dma_start(out=cc_in[:], in_=ar_sb[:])
                nc.gpsimd.collective_compute(
                    "AllReduce",
                    mybir.AluOpType.add,
                    replica_groups=[list(range(NCORES))],
                    ins=[cc_in.opt()],
                    outs=[cc_out.opt()],
                )
                nc.sync.dma_start(out=out_t[:], in_=cc_out[:])
            else:
                # each core ships its partials; host sums during unshard
                nc.sync.dma_start(out=out_t[:], in_=ar_sb[:])

    nc.compile()
    return nc


_CACHE = {}


def _get_program(use_fp32r=True, use_collective=False, id_bf16=False):
    key = (bool(use_fp32r), bool(use_collective), bool(id_bf16))
    if key not in _CACHE:
        _CACHE[key] = _build(*key)
    return _CACHE[key]


def _in_maps(final_result, W, b):
    final_result = np.ascontiguousarray(final_result, dtype=np.float32)
    W = np.ascontiguousarray(W, dtype=np.float32)
    b = np.ascontiguousarray(b, dtype=np.float32).reshape(1, H)
    x0 = np.ascontiguousarray(final_result[0]).reshape(DCH, 128)
    return [
        {
            "x": final_result[c * NSHARD:(c + 1) * NSHARD],
            "w": W,
            "b": b,
            "x0": x0,
        }
        for c in range(NCORES)
    ]


def _finalize(ar):
    ho = ar[:, 0:D]
    z = ar[:, D:D + 1]
    r = (ho / (H * z)).sum(axis=0, dtype=np.float32)
    return np.maximum(r, np.float32(0)).astype(np.float32)


def kernel(final_result, W, b):
    nc = _get_program()
    res = run_bass_kernel_spmd(nc, _in_maps(final_result, W, b),
                               list(range(NCORES)))
    parts = [np.asarray(res.results[c]["out"], dtype=np.float32)
             for c in range(NCORES)]
    return _finalize(np.sum(parts, axis=0, dtype=np.float32))


if __name__ == "__main__":
    rng = np.random.default_rng(0)
    x = rng.standard_normal((N, D), dtype=np.float32)
    W = (rng.standard_normal((H, 2 * D)) * 0.05).astype(np.float32)
    b = (rng.standard_normal(H) * 0.05).astype(np.float32)
    out = kernel(final_result=x, W=W, b=b)
    print("kernel out:", out.shape, out[:8])
